# revision 1
# baseline (speedup 1.0000x reference)
"""KNN block-edge kernel for Trainium2 (8 NeuronCores, one segment per core).

Problem (hardcoded from the reference):
  B=8 segments x NPER=512 blocks x U=4 units, 3-D positions, K=16.
  Candidate edges = all intra-segment block pairs (row-major, C=512 per row).
  Block-block distance = min over the 4x4 unit pairs of Euclidean distance.
  Output = per row the K nearest candidate edges, distance-ascending
  (ties: ascending edge index), as (row_o, col_o, attr) int32 arrays.

Device strategy per core (segment b):
  PE computes -d2(iu, jv) = 2*x.y - |x|^2 - |y|^2 for all 2048x2048 unit
  pairs via a K=5 augmented matmul (64 MMs of [5,128]x[5,512]).  VectorE +
  GpSimd run a max tree to pool the 4x4 unit pairs -> S = -d2min [512,512].
  VectorE extracts the per-row top-16 with max8 / max_index / match_replace
  (monotone in true distance, so sqrt is unnecessary).  Host maps local
  column indices through the actual row/col inputs and repairs the (rare)
  bitwise-tie rows using the shipped S matrix.
"""

import numpy as np

B = 8
NPER = 512
U = 4
KTOP = 16
NU = NPER * U          # units per segment (2048)
NBLK = B * NPER        # total blocks (4096)
MT = NPER // 128       # row tiles per core (4)
NEG_INF = -3.0e38

_cache = {}


def _build_bass():
    import concourse.bacc as bacc
    import concourse.mybir as mybir
    from concourse.tile import TileContext

    f32 = mybir.dt.float32
    u32 = mybir.dt.uint32

    # Bacc (not raw Bass): its compile() pass splits multi-semaphore waits —
    # TRN2 compute instructions carry at most one wait.
    nc = bacc.Bacc("TRN2")
    # Single input tensor (one DMA, one semaphore — LDWEIGHTS has a single
    # wait slot): cols [0, 2048) = lhs columns u-major, cols [2048, 4096) =
    # rhs columns (all units).
    ops = nc.dram_tensor("ops", [5, U * NPER + NU], f32, kind="ExternalInput")
    out_idx = nc.dram_tensor("out_idx", [MT, 128, KTOP], u32, kind="ExternalOutput")
    out_val = nc.dram_tensor("out_val", [MT, 128, KTOP], f32, kind="ExternalOutput")
    out_s = nc.dram_tensor("out_s", [MT, 128, NPER], f32, kind="ExternalOutput")

    with TileContext(nc) as tc:
        with (
            tc.tile_pool(name="const", bufs=1) as cpool,
            tc.tile_pool(name="psum", bufs=1, space="PSUM") as ppool,
            tc.tile_pool(name="work", bufs=2) as wpool,
            tc.tile_pool(name="topk", bufs=2) as kpool,
        ):
            ops_sb = cpool.tile([5, U * NPER + NU], f32)
            nc.sync.dma_start(out=ops_sb, in_=ops[:, :])
            lhs_sb = ops_sb[:, : U * NPER]
            rhs_sb = ops_sb[:, U * NPER:]

            # Two persistent PSUM tensors (4 banks each), ping-ponged across
            # the 16 (t, u) matmul groups.  Pool-slot reuse would attach an
            # extra slot-release wait to the first matmul of each group, and
            # Matmult/LDWEIGHTS has a single wait slot.
            ps_ab = [
                ppool.tile([128, NU], f32, tag="psA", name="psA"),
                ppool.tile([128, NU], f32, tag="psB", name="psB"),
            ]
            for t in range(MT):
                m1 = []
                for u in range(U):
                    ps = ps_ab[(t * U + u) % 2]
                    lv = lhs_sb[:, u * NPER + t * 128: u * NPER + (t + 1) * 128]
                    for c in range(4):
                        nc.tensor.matmul(
                            ps[:, c * 512:(c + 1) * 512],
                            lhsT=lv,
                            rhs=rhs_sb[:, c * 512:(c + 1) * 512],
                            start=True,
                            stop=True,
                        )
                    ps3 = ps.rearrange("p (j v) -> p j v", v=4)
                    m1u = wpool.tile([128, NPER], f32, tag=f"m1{u}", bufs=2)
                    nc.vector.tensor_reduce(
                        m1u, ps3, mybir.AxisListType.X, mybir.AluOpType.max
                    )
                    m1.append(m1u)
                ma = wpool.tile([128, NPER], f32, tag="ma", bufs=2)
                mb = wpool.tile([128, NPER], f32, tag="mb", bufs=2)
                s = wpool.tile([128, NPER], f32, tag="s", bufs=2)
                nc.vector.tensor_max(ma, m1[0], m1[1])
                nc.vector.tensor_max(mb, m1[2], m1[3])
                nc.vector.tensor_max(s, ma, mb)
                nc.sync.dma_start(out=out_s[t], in_=s)

                v8a = kpool.tile([128, 8], f32, tag="v8a")
                i8a = kpool.tile([128, 8], u32, tag="i8a")
                v8b = kpool.tile([128, 8], f32, tag="v8b")
                i8b = kpool.tile([128, 8], u32, tag="i8b")
                s2 = wpool.tile([128, NPER], f32, tag="s2", bufs=2)
                nc.vector.max(out=v8a, in_=s)
                nc.vector.max_index(out=i8a, in_max=v8a, in_values=s)
                nc.vector.match_replace(
                    out=s2, in_to_replace=v8a, in_values=s, imm_value=NEG_INF
                )
                nc.vector.max(out=v8b, in_=s2)
                nc.vector.max_index(out=i8b, in_max=v8b, in_values=s2)

                nc.sync.dma_start(out=out_val[t][:, 0:8], in_=v8a)
                nc.sync.dma_start(out=out_val[t][:, 8:16], in_=v8b)
                nc.sync.dma_start(out=out_idx[t][:, 0:8], in_=i8a)
                nc.sync.dma_start(out=out_idx[t][:, 8:16], in_=i8b)
    nc.compile()
    return nc


def _get_nc():
    if "nc" not in _cache:
        _cache["nc"] = _build_bass()
    return _cache["nc"]


def _make_core_inputs(unit_pos):
    """Per-core augmented operands.  Core b handles segment b.

    rhs columns (unit a): [x0, x1, x2, 1, n_a]
    lhs columns (block i, unit u): [2*x0, 2*x1, 2*x2, -n_iu, -1]
    so that lhs.col . rhs.col = 2 x.y - n_i - n_j = -d2.
    """
    in_maps = []
    for b in range(B):
        P = np.ascontiguousarray(unit_pos[b * NU:(b + 1) * NU]).astype(
            np.float32, copy=False
        )
        n = (P * P).sum(axis=1, dtype=np.float32).astype(np.float32)
        ops = np.empty((5, U * NPER + NU), np.float32)
        for u in range(U):
            sl = slice(u * NPER, (u + 1) * NPER)
            X = P[u::U]
            ops[0:3, sl] = (np.float32(2.0) * X).T
            ops[3, sl] = -n[u::U]
            ops[4, sl] = -1.0
        rsl = slice(U * NPER, None)
        ops[0:3, rsl] = P.T
        ops[3, rsl] = 1.0
        ops[4, rsl] = n
        in_maps.append({"ops": ops})
    return in_maps


def _run_device(in_maps, trace=False):
    from concourse.bass_utils import run_bass_kernel_spmd

    nc = _get_nc()
    res = run_bass_kernel_spmd(
        nc, in_maps, core_ids=list(range(B)), trace=trace
    )
    return res


def _topk_from_scores(s_row):
    """Reference-exact per-row top-K from the score row (s = -d2min):
    ascending d2, ties by ascending local index."""
    order = np.argsort(-s_row, kind="stable")[:KTOP]
    return order


def _postprocess(results, row, col):
    """Map device top-k (local j, per segment) to (row_o, col_o, attr)."""
    row_mat = row.reshape(NBLK, NPER)
    col_mat = col.reshape(NBLK, NPER)
    row_o = np.empty((NBLK, KTOP), np.int32)
    col_o = np.empty((NBLK, KTOP), np.int32)
    for b in range(B):
        r = results[b]
        idx = r["out_idx"].reshape(NPER, KTOP).astype(np.int64)
        val = r["out_val"].reshape(NPER, KTOP)
        smat = r["out_s"].reshape(NPER, NPER)

        # Repair rows where a bitwise-equal score appears more than once in
        # the top-16: max_index returns the first occurrence for every equal
        # value, so such rows show duplicate indices.  Re-derive those rows
        # from the on-device score matrix with reference tie semantics.
        dup = (np.sort(idx, axis=1)[:, 1:] == np.sort(idx, axis=1)[:, :-1]).any(axis=1)
        # Also guard against non-monotone value order (shouldn't happen).
        nonmono = (np.diff(val, axis=1) > 0).any(axis=1)
        for rloc in np.flatnonzero(dup | nonmono):
            idx[rloc] = _topk_from_scores(smat[rloc])

        gr = slice(b * NPER, (b + 1) * NPER)
        rows_local = np.arange(NPER)[:, None]
        row_o[gr] = row_mat[gr][rows_local, idx]
        col_o[gr] = col_mat[gr][rows_local, idx]
    attr = np.zeros(NBLK * KTOP, np.int32)
    return row_o.reshape(-1), col_o.reshape(-1), attr


def kernel(unit_pos, row, col, unit2block, segment_ids, k):
    unit_pos = np.asarray(unit_pos, dtype=np.float32)
    row = np.asarray(row, dtype=np.int32)
    col = np.asarray(col, dtype=np.int32)
    assert int(k) == KTOP
    in_maps = _make_core_inputs(unit_pos)
    res = _run_device(in_maps, trace=False)
    return _postprocess(res.results, row, col)



# revision 2
# speedup vs baseline: 2.0579x; 2.0579x over previous
"""KNN block-edge kernel for Trainium2 (8 NeuronCores, one segment per core).

Problem (hardcoded from the reference):
  B=8 segments x NPER=512 blocks x U=4 units, 3-D positions, K=16.
  Candidate edges = all intra-segment block pairs (row-major, C=512 per row).
  Block-block distance = min over the 4x4 unit pairs of Euclidean distance.
  Output = per row the K nearest candidate edges, distance-ascending
  (ties: ascending edge index), as (row_o, col_o, attr) int32 arrays.

Device strategy per core (segment b):
  PE computes -d2(iu, jv) = 2*x.y - |x|^2 - |y|^2 for all 2048x2048 unit
  pairs.  fp32 matmuls on the PE run as slow LOW/HIGH pairs, so instead the
  operands are split into fp16 hi+lo halves and all cross terms are stacked
  along the contract dim: one K=16 fp16 matmul per tile (product error
  ~|v|*2^-22, on par with the fp32 path).  Matmuls for a (row-tile t, col
  chunk c) pair land u-major in one 4-bank PSUM tile; a single VectorE
  tensor_reduce(max, XY) then folds all 16 (u,v) channels at once, giving
  S = -d2min [512,512] with no separate max tree.  VectorE extracts the
  per-row top-16 with max8 / max_index / match_replace (monotone in true
  distance, so sqrt is unnecessary).  Host maps local column indices through
  the actual row/col inputs and repairs the (rare) bitwise-tie rows using
  the shipped S matrix.
"""

import numpy as np

B = 8
NPER = 512
U = 4
KTOP = 16
NU = NPER * U          # units per segment (2048)
NBLK = B * NPER        # total blocks (4096)
MT = NPER // 128       # row tiles per core (4)
KC = 16                # contract dim of the split fp16 matmul
NEG_INF = -3.0e38

_cache = {}


def _build_bass():
    import concourse.bacc as bacc
    import concourse.mybir as mybir
    from concourse.tile import TileContext

    f16 = mybir.dt.float16
    f32 = mybir.dt.float32
    u32 = mybir.dt.uint32

    # Bacc (not raw Bass): its compile() pass splits multi-semaphore waits —
    # TRN2 compute instructions carry at most one wait.
    nc = bacc.Bacc("TRN2")
    # Single input tensor (one DMA, one semaphore — LDWEIGHTS has a single
    # wait slot): cols [0, 2048) = lhs columns u-major, cols [2048, 4096) =
    # rhs columns (all units).
    ops = nc.dram_tensor("ops", [KC, U * NPER + NU], f16, kind="ExternalInput")
    out_idx = nc.dram_tensor("out_idx", [MT, 128, KTOP], u32, kind="ExternalOutput")
    out_val = nc.dram_tensor("out_val", [MT, 128, KTOP], f32, kind="ExternalOutput")
    out_s = nc.dram_tensor("out_s", [MT, 128, NPER], f32, kind="ExternalOutput")

    with TileContext(nc) as tc:
        with (
            tc.tile_pool(name="const", bufs=1) as cpool,
            tc.tile_pool(name="psum", bufs=1, space="PSUM") as ppool,
            tc.tile_pool(name="work", bufs=2) as wpool,
            tc.tile_pool(name="topk", bufs=2) as kpool,
        ):
            ops_sb = cpool.tile([KC, U * NPER + NU], f16)
            nc.sync.dma_start(out=ops_sb, in_=ops[:, :])
            lhs_sb = ops_sb[:, : U * NPER]
            rhs_sb = ops_sb[:, U * NPER:]

            # Two persistent PSUM tensors (4 banks each), ping-ponged across
            # the 16 (t, c) tile pairs.  Pool-slot reuse would attach an
            # extra slot-release wait to the first matmul of each group, and
            # Matmult/LDWEIGHTS has a single wait slot.
            ps_ab = [
                ppool.tile([128, NU], f32, tag="psA", name="psA"),
                ppool.tile([128, NU], f32, tag="psB", name="psB"),
            ]
            for t in range(MT):
                s = wpool.tile([128, NPER], f32, tag="s", bufs=2)
                for c in range(MT):
                    ps = ps_ab[(t * MT + c) % 2]
                    for u in range(U):
                        nc.tensor.matmul(
                            ps[:, u * 512:(u + 1) * 512],
                            lhsT=lhs_sb[:, u * NPER + t * 128: u * NPER + (t + 1) * 128],
                            rhs=rhs_sb[:, c * 512:(c + 1) * 512],
                            start=True,
                            stop=True,
                        )
                    # [128, (u j v)] -> max over u and v in one pass
                    ps4 = ps.rearrange("p (u j v) -> p j u v", u=U, v=U)
                    nc.vector.tensor_reduce(
                        s[:, c * 128:(c + 1) * 128], ps4,
                        mybir.AxisListType.XY, mybir.AluOpType.max,
                    )
                nc.sync.dma_start(out=out_s[t], in_=s)

                v8a = kpool.tile([128, 8], f32, tag="v8a")
                i8a = kpool.tile([128, 8], u32, tag="i8a")
                v8b = kpool.tile([128, 8], f32, tag="v8b")
                i8b = kpool.tile([128, 8], u32, tag="i8b")
                s2 = wpool.tile([128, NPER], f32, tag="s2", bufs=2)
                nc.vector.max(out=v8a, in_=s)
                nc.vector.max_index(out=i8a, in_max=v8a, in_values=s)
                nc.vector.match_replace(
                    out=s2, in_to_replace=v8a, in_values=s, imm_value=NEG_INF
                )
                nc.vector.max(out=v8b, in_=s2)
                nc.vector.max_index(out=i8b, in_max=v8b, in_values=s2)

                nc.sync.dma_start(out=out_val[t][:, 0:8], in_=v8a)
                nc.sync.dma_start(out=out_val[t][:, 8:16], in_=v8b)
                nc.sync.dma_start(out=out_idx[t][:, 0:8], in_=i8a)
                nc.sync.dma_start(out=out_idx[t][:, 8:16], in_=i8b)
    nc.compile()
    return nc


def _get_nc():
    if "nc" not in _cache:
        _cache["nc"] = _build_bass()
    return _cache["nc"]


def _split16(v):
    h = v.astype(np.float16)
    l = (v - h.astype(np.float32)).astype(np.float16)
    return h, l


def _make_core_inputs(unit_pos):
    """Per-core fp16 split operands.  Core b handles segment b.

    Contract rows (lhs col = block i unit u; rhs col = unit (j,v)):
      0-2 : 2x_hi . y_hi     3-5 : 2x_hi . y_lo
      6-8 : 2x_lo . y_hi     9-11: 2x_lo . y_lo
      12  : -n_hi . 1        13  : -n_lo . 1
      14  : -1 . n_hi        15  : -1 . n_lo
    so that lhs.col . rhs.col = 2 x.y - n_i - n_j = -d2 exactly to ~2^-22.
    """
    in_maps = []
    for b in range(B):
        P = np.ascontiguousarray(unit_pos[b * NU:(b + 1) * NU]).astype(
            np.float32, copy=False
        )
        n64 = (P.astype(np.float64) ** 2).sum(axis=1)
        ah, al = _split16((2.0 * P.astype(np.float64)).astype(np.float32))
        yh, yl = _split16(P)
        nh = n64.astype(np.float16)
        nl = (n64 - nh.astype(np.float64)).astype(np.float16)

        ops = np.empty((KC, U * NPER + NU), np.float16)
        for u in range(U):
            sl = slice(u * NPER, (u + 1) * NPER)
            ops[0:3, sl] = ah[u::U].T
            ops[3:6, sl] = ah[u::U].T
            ops[6:9, sl] = al[u::U].T
            ops[9:12, sl] = al[u::U].T
            ops[12, sl] = -nh[u::U]
            ops[13, sl] = -nl[u::U]
            ops[14, sl] = -1.0
            ops[15, sl] = -1.0
        rsl = slice(U * NPER, None)
        ops[0:3, rsl] = yh.T
        ops[3:6, rsl] = yl.T
        ops[6:9, rsl] = yh.T
        ops[9:12, rsl] = yl.T
        ops[12, rsl] = 1.0
        ops[13, rsl] = 1.0
        ops[14, rsl] = nh
        ops[15, rsl] = nl
        in_maps.append({"ops": ops})
    return in_maps


def _run_device(in_maps, trace=False):
    from concourse.bass_utils import run_bass_kernel_spmd

    nc = _get_nc()
    res = run_bass_kernel_spmd(
        nc, in_maps, core_ids=list(range(B)), trace=trace
    )
    return res


def _topk_from_scores(s_row):
    """Reference-exact per-row top-K from the score row (s = -d2min):
    ascending d2, ties by ascending local index."""
    order = np.argsort(-s_row, kind="stable")[:KTOP]
    return order


def _postprocess(results, row, col):
    """Map device top-k (local j, per segment) to (row_o, col_o, attr)."""
    row_mat = row.reshape(NBLK, NPER)
    col_mat = col.reshape(NBLK, NPER)
    row_o = np.empty((NBLK, KTOP), np.int32)
    col_o = np.empty((NBLK, KTOP), np.int32)
    for b in range(B):
        r = results[b]
        idx = r["out_idx"].reshape(NPER, KTOP).astype(np.int64)
        val = r["out_val"].reshape(NPER, KTOP)
        smat = r["out_s"].reshape(NPER, NPER)

        # Repair rows where a bitwise-equal score appears more than once in
        # the top-16: max_index returns the first occurrence for every equal
        # value, so such rows show duplicate indices.  Re-derive those rows
        # from the on-device score matrix with reference tie semantics.
        dup = (np.sort(idx, axis=1)[:, 1:] == np.sort(idx, axis=1)[:, :-1]).any(axis=1)
        # Also guard against non-monotone value order (shouldn't happen).
        nonmono = (np.diff(val, axis=1) > 0).any(axis=1)
        for rloc in np.flatnonzero(dup | nonmono):
            idx[rloc] = _topk_from_scores(smat[rloc])

        gr = slice(b * NPER, (b + 1) * NPER)
        rows_local = np.arange(NPER)[:, None]
        row_o[gr] = row_mat[gr][rows_local, idx]
        col_o[gr] = col_mat[gr][rows_local, idx]
    attr = np.zeros(NBLK * KTOP, np.int32)
    return row_o.reshape(-1), col_o.reshape(-1), attr


def kernel(unit_pos, row, col, unit2block, segment_ids, k):
    unit_pos = np.asarray(unit_pos, dtype=np.float32)
    row = np.asarray(row, dtype=np.int32)
    col = np.asarray(col, dtype=np.int32)
    assert int(k) == KTOP
    in_maps = _make_core_inputs(unit_pos)
    res = _run_device(in_maps, trace=False)
    return _postprocess(res.results, row, col)


# revision 5
# speedup vs baseline: 2.3341x; 1.1342x over previous
"""KNN block-edge kernel for Trainium2 (8 NeuronCores, one segment per core).

Problem (hardcoded from the reference):
  B=8 segments x NPER=512 blocks x U=4 units, 3-D positions, K=16.
  Candidate edges = all intra-segment block pairs (row-major, C=512 per row).
  Block-block distance = min over the 4x4 unit pairs of Euclidean distance.
  Output = per row the K nearest candidate edges, distance-ascending
  (ties: ascending edge index), as (row_o, col_o, attr) int32 arrays.

Device strategy per core (segment b):
  PE computes -d2(iu, jv) = 2*x.y - |x|^2 - |y|^2 for unit pairs.  fp32
  matmuls on the PE run as slow LOW/HIGH pairs, so operands are split into
  fp16 hi+lo halves with all cross terms stacked along the contract dim:
  one K=16 fp16 matmul per tile (product error ~|v|*2^-22, on par with the
  fp32 path).  Matmuls for a (row-tile t, col-tile c) pair land u-major in
  one 4-bank PSUM tile; a single VectorE tensor_reduce(max, XY) folds all
  16 (u,v) channels at once, producing the S = -d2min chunk [128,128].
  S is symmetric, so only the 10 upper-triangle (t<=c) chunks are computed;
  the 6 mirrors are PE-transposed out of SBUF and evacuated by the (idle)
  ScalarE.  During the input DMA the PE runs a warmup burst on zeroed
  operands so the HAM clock-gate is at 2.4 GHz when real tiles start.
  VectorE extracts the per-row top-16 with max8 / max_index / match_replace
  (monotone in true distance, so sqrt is unnecessary).  Host maps local
  column indices through the actual row/col inputs and repairs the (rare)
  bitwise-tie rows using the shipped upper-triangle S pieces.
"""

import numpy as np

B = 8
NPER = 512
U = 4
KTOP = 16
NU = NPER * U          # units per segment (2048)
NBLK = B * NPER        # total blocks (4096)
MT = NPER // 128       # row tiles per core (4)
KC = 16                # contract dim of the split fp16 matmul
NEG_INF = -3.0e38
# ops column layout: [lhs t=0 (512) | rhs (2048) | lhs t=1..3 (1536)]
RHS0 = 512
LHS1 = RHS0 + NU

_cache = {}


def _lhs_col(t, u):
    return u * 128 if t == 0 else LHS1 + (t - 1) * 512 + u * 128


def _build_bass():
    import concourse.bacc as bacc
    import concourse.mybir as mybir
    from concourse.tile import TileContext

    f16 = mybir.dt.float16
    f32 = mybir.dt.float32
    u32 = mybir.dt.uint32

    # Bacc (not raw Bass): its compile() pass splits multi-semaphore waits —
    # TRN2 compute instructions carry at most one wait.
    nc = bacc.Bacc("TRN2")
    ops = nc.dram_tensor("ops", [KC, U * NPER + NU], f16, kind="ExternalInput")
    ident = nc.dram_tensor("ident", [128, 128], f32, kind="ExternalInput")
    out_idx = nc.dram_tensor("out_idx", [MT, 128, KTOP], u32, kind="ExternalOutput")
    out_val = nc.dram_tensor("out_val", [MT, 128, KTOP], f32, kind="ExternalOutput")
    out_s = nc.dram_tensor("out_s", [MT, 128, NPER], f32, kind="ExternalOutput")

    with TileContext(nc) as tc:
        with (
            tc.tile_pool(name="const", bufs=1) as cpool,
            tc.tile_pool(name="psum", bufs=1, space="PSUM") as ppool,
            tc.tile_pool(name="work", bufs=2) as wpool,
            tc.tile_pool(name="topk", bufs=2) as kpool,
        ):
            # -- warmup: zeroed fp16 operands, matmuls into psB scratch while
            # the real input DMA is in flight (flips PE HAM to 2.4 GHz).
            wl = cpool.tile([KC, 128], f16)
            wr = cpool.tile([KC, 512], f16)
            nc.gpsimd.memset(wl, 0.0)
            nc.gpsimd.memset(wr, 0.0)
            warm_dummy = cpool.tile([1, 8], f32)

            ops_sb = cpool.tile([KC, U * NPER + NU], f16)
            ident_sb = cpool.tile([128, 128], f32)
            # Phase-0 operands (lhs t=0 + all rhs) first, rest second.
            nc.sync.dma_start(out=ops_sb[:, :LHS1], in_=ops[:, :LHS1])
            nc.sync.dma_start(out=ops_sb[:, LHS1:], in_=ops[:, LHS1:])
            nc.sync.dma_start(out=ident_sb, in_=ident[:, :])
            rhs_sb = ops_sb[:, RHS0:RHS0 + NU]

            ps_ab = [
                ppool.tile([128, NU], f32, tag="psA", name="psA"),
                ppool.tile([128, NU], f32, tag="psB", name="psB"),
            ]
            for _ in range(8):
                nc.tensor.matmul(ps_ab[1][:, 0:512], lhsT=wl, rhs=wr,
                                 start=True, stop=True)
            # Trigger the ACT Copy table load early (overlaps input DMA).
            nc.scalar.copy(out=warm_dummy, in_=wl[0:1, 0:8])

            # Persistent per-tile score rows; mirrors land here via ScalarE.
            s_rows = [cpool.tile([128, NPER], f32, name=f"s{t}") for t in range(MT)]

            n = 0  # direct (upper-triangle) chunk counter
            for t in range(MT):
                s = s_rows[t]
                n_first = n
                for c in range(t, MT):
                    ps = ps_ab[n % 2]
                    for u in range(U):
                        nc.tensor.matmul(
                            ps[:, u * 512:(u + 1) * 512],
                            lhsT=ops_sb[:, _lhs_col(t, u):_lhs_col(t, u) + 128],
                            rhs=rhs_sb[:, c * 512:(c + 1) * 512],
                            start=True,
                            stop=True,
                        )
                    # [128, (u j v)] -> max over u and v in one pass
                    ps4 = ps.rearrange("p (u j v) -> p j u v", u=U, v=U)
                    nc.vector.tensor_reduce(
                        s[:, c * 128:(c + 1) * 128], ps4,
                        mybir.AxisListType.XY, mybir.AluOpType.max,
                    )
                    nc.sync.dma_start(out=out_s[t][:, c * 128:(c + 1) * 128],
                                      in_=s[:, c * 128:(c + 1) * 128])
                    n += 1
                # Mirrors for the NEXT tiles: transpose this tile's freshly
                # reduced chunks (t, c>t) into s_rows[c][:, t*128:...].
                # PSUM scratch: tail columns of the next direct chunk's tile —
                # its pending reduce is the oldest, so the transpose doesn't
                # stall the PE, and only that chunk's u=3 matmul overlaps
                # (waits for the ScalarE evacuation).
                mtile = ps_ab[n % 2]
                for k, c in enumerate(range(t + 1, MT)):
                    sl = slice(1536 + k * 128, 1536 + (k + 1) * 128)
                    nc.tensor.transpose(
                        mtile[:, sl], in_=s[:, c * 128:(c + 1) * 128],
                        identity=ident_sb,
                    )
                    nc.scalar.copy(out=s_rows[c][:, t * 128:(t + 1) * 128],
                                   in_=mtile[:, sl])

                v8a = kpool.tile([128, 8], f32, tag="v8a")
                i8a = kpool.tile([128, 8], u32, tag="i8a")
                v8b = kpool.tile([128, 8], f32, tag="v8b")
                i8b = kpool.tile([128, 8], u32, tag="i8b")
                s2 = wpool.tile([128, NPER], f32, tag="s2", bufs=2)
                nc.vector.max(out=v8a, in_=s)
                nc.vector.max_index(out=i8a, in_max=v8a, in_values=s)
                nc.vector.match_replace(
                    out=s2, in_to_replace=v8a, in_values=s, imm_value=NEG_INF
                )
                nc.vector.max(out=v8b, in_=s2)
                nc.vector.max_index(out=i8b, in_max=v8b, in_values=s2)

                nc.sync.dma_start(out=out_val[t][:, 0:8], in_=v8a)
                nc.sync.dma_start(out=out_val[t][:, 8:16], in_=v8b)
                nc.sync.dma_start(out=out_idx[t][:, 0:8], in_=i8a)
                nc.sync.dma_start(out=out_idx[t][:, 8:16], in_=i8b)
    nc.compile()
    return nc


def _get_nc():
    if "nc" not in _cache:
        _cache["nc"] = _build_bass()
    return _cache["nc"]


def _split16(v):
    h = v.astype(np.float16)
    l = (v - h.astype(np.float32)).astype(np.float16)
    return h, l


def _make_core_inputs(unit_pos):
    """Per-core fp16 split operands.  Core b handles segment b.

    Contract rows (lhs col = block i unit u; rhs col = unit (j,v)):
      0-2 : 2x_hi . y_hi     3-5 : 2x_hi . y_lo
      6-8 : 2x_lo . y_hi     9-11: 2x_lo . y_lo
      12  : -n_hi . 1        13  : -n_lo . 1
      14  : -1 . n_hi        15  : -1 . n_lo
    so that lhs.col . rhs.col = 2 x.y - n_i - n_j = -d2 exactly to ~2^-22.
    """
    ident = np.eye(128, dtype=np.float32)
    in_maps = []
    for b in range(B):
        P = np.ascontiguousarray(unit_pos[b * NU:(b + 1) * NU]).astype(
            np.float32, copy=False
        )
        n64 = (P.astype(np.float64) ** 2).sum(axis=1)
        ah, al = _split16((2.0 * P.astype(np.float64)).astype(np.float32))
        yh, yl = _split16(P)
        nh = n64.astype(np.float16)
        nl = (n64 - nh.astype(np.float64)).astype(np.float16)

        lhs = np.empty((KC, NPER * U), np.float16)  # u-major, reordered below
        for u in range(U):
            sl = slice(u * NPER, (u + 1) * NPER)
            lhs[0:3, sl] = ah[u::U].T
            lhs[3:6, sl] = ah[u::U].T
            lhs[6:9, sl] = al[u::U].T
            lhs[9:12, sl] = al[u::U].T
            lhs[12, sl] = -nh[u::U]
            lhs[13, sl] = -nl[u::U]
            lhs[14, sl] = -1.0
            lhs[15, sl] = -1.0
        # reorder to [t, u, i]: col t*512 + u*128 + i  <-  u*512 + t*128 + i
        lhs_tu = lhs.reshape(KC, U, MT, 128).transpose(0, 2, 1, 3).reshape(KC, -1)

        ops = np.empty((KC, U * NPER + NU), np.float16)
        ops[:, :RHS0] = lhs_tu[:, :512]
        ops[:, LHS1:] = lhs_tu[:, 512:]
        rsl = slice(RHS0, LHS1)
        ops[0:3, rsl] = yh.T
        ops[3:6, rsl] = yl.T
        ops[6:9, rsl] = yh.T
        ops[9:12, rsl] = yl.T
        ops[12, rsl] = 1.0
        ops[13, rsl] = 1.0
        ops[14, rsl] = nh
        ops[15, rsl] = nl
        in_maps.append({"ops": ops, "ident": ident})
    return in_maps


def _run_device(in_maps, trace=False):
    from concourse.bass_utils import run_bass_kernel_spmd

    nc = _get_nc()
    res = run_bass_kernel_spmd(
        nc, in_maps, core_ids=list(range(B)), trace=trace
    )
    return res


def _topk_from_scores(s_row):
    """Reference-exact per-row top-K from the score row (s = -d2min):
    ascending d2, ties by ascending local index."""
    order = np.argsort(-s_row, kind="stable")[:KTOP]
    return order


def _assemble_s(smat):
    """Full [512,512] S from the 10 shipped upper-triangle pieces."""
    S = np.empty((NPER, NPER), np.float32)
    for t in range(MT):
        rt = slice(t * 128, (t + 1) * 128)
        for c in range(t, MT):
            rc = slice(c * 128, (c + 1) * 128)
            piece = smat[t][:, rc]
            S[rt, rc] = piece
            if c != t:
                S[rc, rt] = piece.T
    return S


def _postprocess(results, row, col):
    """Map device top-k (local j, per segment) to (row_o, col_o, attr)."""
    row_mat = row.reshape(NBLK, NPER)
    col_mat = col.reshape(NBLK, NPER)
    row_o = np.empty((NBLK, KTOP), np.int32)
    col_o = np.empty((NBLK, KTOP), np.int32)
    for b in range(B):
        r = results[b]
        idx = r["out_idx"].reshape(NPER, KTOP).astype(np.int64)
        val = r["out_val"].reshape(NPER, KTOP)

        # Repair rows where a bitwise-equal score appears more than once in
        # the top-16: max_index returns the first occurrence for every equal
        # value, so such rows show duplicate indices.  Re-derive those rows
        # from the on-device score matrix with reference tie semantics.
        dup = (np.sort(idx, axis=1)[:, 1:] == np.sort(idx, axis=1)[:, :-1]).any(axis=1)
        # Also guard against non-monotone value order (shouldn't happen).
        nonmono = (np.diff(val, axis=1) > 0).any(axis=1)
        bad = np.flatnonzero(dup | nonmono)
        if bad.size:
            S = _assemble_s(r["out_s"].reshape(MT, 128, NPER))
            for rloc in bad:
                idx[rloc] = _topk_from_scores(S[rloc])

        gr = slice(b * NPER, (b + 1) * NPER)
        rows_local = np.arange(NPER)[:, None]
        row_o[gr] = row_mat[gr][rows_local, idx]
        col_o[gr] = col_mat[gr][rows_local, idx]
    attr = np.zeros(NBLK * KTOP, np.int32)
    return row_o.reshape(-1), col_o.reshape(-1), attr


def kernel(unit_pos, row, col, unit2block, segment_ids, k):
    unit_pos = np.asarray(unit_pos, dtype=np.float32)
    row = np.asarray(row, dtype=np.int32)
    col = np.asarray(col, dtype=np.int32)
    assert int(k) == KTOP
    in_maps = _make_core_inputs(unit_pos)
    res = _run_device(in_maps, trace=False)
    return _postprocess(res.results, row, col)


# revision 8
# speedup vs baseline: 2.5086x; 1.0748x over previous
"""KNN block-edge kernel for Trainium2 (8 NeuronCores, one segment per core).

Problem (hardcoded from the reference):
  B=8 segments x NPER=512 blocks x U=4 units, 3-D positions, K=16.
  Candidate edges = all intra-segment block pairs (row-major, C=512 per row).
  Block-block distance = min over the 4x4 unit pairs of Euclidean distance.
  Output = per row the K nearest candidate edges, distance-ascending
  (ties: ascending edge index), as (row_o, col_o, attr) int32 arrays.

Device strategy per core (segment b):
  PE computes -d2(iu, jv) = 2*x.y - |x|^2 - |y|^2 for unit pairs.  fp32
  matmuls on the PE run as slow LOW/HIGH pairs, so operands are split into
  fp16 hi+lo halves with all cross terms stacked along the contract dim:
  one K=16 fp16 matmul per tile (product error ~|v|*2^-22, on par with the
  fp32 path).  Matmuls for a (row-tile t, col-tile c) pair land u-major in
  one 4-bank PSUM tile; a single VectorE tensor_reduce(max, XY) folds all
  16 (u,v) channels at once, producing the S = -d2min chunk [128,128].
  S is symmetric, so only the 10 upper-triangle (t<=c) chunks are computed;
  the 6 mirrors are PE-transposed out of SBUF and evacuated by the (idle)
  ScalarE.  During the input DMA the PE runs a warmup burst on zeroed
  operands so the HAM clock-gate is at 2.4 GHz when real tiles start.
  VectorE extracts the per-row top-16 with max8 / max_index / match_replace
  (monotone in true distance, so sqrt is unnecessary).  Host maps local
  column indices through the actual row/col inputs and repairs the (rare)
  bitwise-tie rows using the shipped upper-triangle S pieces.
"""

import numpy as np

B = 8
NPER = 512
U = 4
KTOP = 16
NU = NPER * U          # units per segment (2048)
NBLK = B * NPER        # total blocks (4096)
MT = NPER // 128       # row tiles per core (4)
KC = 16                # contract dim of the split fp16 matmul
NEG_INF = -3.0e38
# ops column layout: [lhs t=0 (512) | rhs (2048) | lhs t=1..3 (1536)]
RHS0 = 512
LHS1 = RHS0 + NU

_cache = {}


def _lhs_col(t, u):
    return u * 128 if t == 0 else LHS1 + (t - 1) * 512 + u * 128


def _build_bass():
    import concourse.bacc as bacc
    import concourse.mybir as mybir
    from concourse.tile import TileContext

    f16 = mybir.dt.float16
    f32 = mybir.dt.float32
    u32 = mybir.dt.uint32

    # Bacc (not raw Bass): its compile() pass splits multi-semaphore waits —
    # TRN2 compute instructions carry at most one wait.
    nc = bacc.Bacc("TRN2")
    ops = nc.dram_tensor("ops", [KC, U * NPER + NU], f16, kind="ExternalInput")
    ident = nc.dram_tensor("ident", [128, 128], f32, kind="ExternalInput")
    out_idx = nc.dram_tensor("out_idx", [MT, 128, KTOP], u32, kind="ExternalOutput")
    out_val = nc.dram_tensor("out_val", [MT, 128, KTOP], f32, kind="ExternalOutput")

    with TileContext(nc) as tc:
        with (
            tc.tile_pool(name="const", bufs=1) as cpool,
            tc.tile_pool(name="psum", bufs=1, space="PSUM") as ppool,
            tc.tile_pool(name="work", bufs=2) as wpool,
            tc.tile_pool(name="topk", bufs=2) as kpool,
        ):
            # -- warmup: zeroed fp16 operands, matmuls into psB scratch while
            # the real input DMA is in flight (flips PE HAM to 2.4 GHz).
            wl = cpool.tile([KC, 128], f16)
            wr = cpool.tile([KC, 512], f16)
            nc.gpsimd.memset(wl, 0.0)
            nc.gpsimd.memset(wr, 0.0)
            warm_dummy = cpool.tile([1, 8], f32)

            ops_sb = cpool.tile([KC, U * NPER + NU], f16)
            ident_sb = cpool.tile([128, 128], f32)
            # Phase-0 operands (lhs t=0 + all rhs) first, rest second.
            nc.sync.dma_start(out=ops_sb[:, :LHS1], in_=ops[:, :LHS1])
            nc.sync.dma_start(out=ops_sb[:, LHS1:], in_=ops[:, LHS1:])
            nc.sync.dma_start(out=ident_sb, in_=ident[:, :])
            rhs_sb = ops_sb[:, RHS0:RHS0 + NU]

            ps_ab = [
                ppool.tile([128, NU], f32, tag="psA", name="psA"),
                ppool.tile([128, NU], f32, tag="psB", name="psB"),
            ]
            for _ in range(8):
                nc.tensor.matmul(ps_ab[1][:, 0:512], lhsT=wl, rhs=wr,
                                 start=True, stop=True)
            # Trigger the ACT Copy table load early (overlaps input DMA).
            nc.scalar.copy(out=warm_dummy, in_=wl[0:1, 0:8])

            # Persistent per-tile score rows; mirrors land here via ScalarE.
            s_rows = [cpool.tile([128, NPER], f32, name=f"s{t}") for t in range(MT)]

            # Upper-triangle chunks in phase order; mirror transposes are
            # interleaved after the MM group of chunk n+2 so the PE never
            # gains a wait it doesn't already have: the transpose scratch is
            # tile (m+1)%2, whose reduce(m-1) the next MM group waits on
            # anyway.  Only that group's u=3 matmul overlaps the scratch
            # columns (waits for the ScalarE evacuation).
            chunks = [(t, c) for t in range(MT) for c in range(t, MT)]
            emit_after = {m: [] for m in range(len(chunks))}
            for n, (t, c) in enumerate(chunks):
                if c > t:
                    emit_after[min(n + 2, len(chunks) - 1)].append(n)

            mir = 0  # rotating scratch column slot
            for n, (t, c) in enumerate(chunks):
                s = s_rows[t]
                ps = ps_ab[n % 2]
                for u in range(U):
                    nc.tensor.matmul(
                        ps[:, u * 512:(u + 1) * 512],
                        lhsT=ops_sb[:, _lhs_col(t, u):_lhs_col(t, u) + 128],
                        rhs=rhs_sb[:, c * 512:(c + 1) * 512],
                        start=True,
                        stop=True,
                    )
                for nsrc in emit_after[n]:
                    ts_, cs_ = chunks[nsrc]
                    mtile = ps_ab[(n + 1) % 2]
                    sl = slice(1536 + (mir % 3) * 128, 1536 + (mir % 3 + 1) * 128)
                    mir += 1
                    nc.tensor.transpose(
                        mtile[:, sl],
                        in_=s_rows[ts_][:, cs_ * 128:(cs_ + 1) * 128],
                        identity=ident_sb,
                    )
                    nc.scalar.copy(out=s_rows[cs_][:, ts_ * 128:(ts_ + 1) * 128],
                                   in_=mtile[:, sl])
                # [128, (u j v)] -> max over u and v in one pass
                ps4 = ps.rearrange("p (u j v) -> p j u v", u=U, v=U)
                nc.vector.tensor_reduce(
                    s[:, c * 128:(c + 1) * 128], ps4,
                    mybir.AxisListType.XY, mybir.AluOpType.max,
                )
                if c < MT - 1:
                    continue
                # last chunk of row-tile t: extract its top-16
                v8a = kpool.tile([128, 8], f32, tag="v8a")
                i8a = kpool.tile([128, 8], u32, tag="i8a")
                v8b = kpool.tile([128, 8], f32, tag="v8b")
                i8b = kpool.tile([128, 8], u32, tag="i8b")
                s2 = wpool.tile([128, NPER], f32, tag="s2", bufs=2)
                nc.vector.max(out=v8a, in_=s)
                nc.vector.max_index(out=i8a, in_max=v8a, in_values=s)
                nc.vector.match_replace(
                    out=s2, in_to_replace=v8a, in_values=s, imm_value=NEG_INF
                )
                nc.vector.max(out=v8b, in_=s2)
                nc.vector.max_index(out=i8b, in_max=v8b, in_values=s2)

                nc.sync.dma_start(out=out_val[t][:, 0:8], in_=v8a)
                nc.sync.dma_start(out=out_val[t][:, 8:16], in_=v8b)
                nc.sync.dma_start(out=out_idx[t][:, 0:8], in_=i8a)
                nc.sync.dma_start(out=out_idx[t][:, 8:16], in_=i8b)
    nc.compile()
    return nc


def _get_nc():
    if "nc" not in _cache:
        _cache["nc"] = _build_bass()
    return _cache["nc"]


def _split16(v):
    h = v.astype(np.float16)
    l = (v - h.astype(np.float32)).astype(np.float16)
    return h, l


def _make_core_inputs(unit_pos):
    """Per-core fp16 split operands.  Core b handles segment b.

    Contract rows (lhs col = block i unit u; rhs col = unit (j,v)):
      0-2 : 2x_hi . y_hi     3-5 : 2x_hi . y_lo
      6-8 : 2x_lo . y_hi     9-11: 2x_lo . y_lo
      12  : -n_hi . 1        13  : -n_lo . 1
      14  : -1 . n_hi        15  : -1 . n_lo
    so that lhs.col . rhs.col = 2 x.y - n_i - n_j = -d2 exactly to ~2^-22.
    """
    ident = np.eye(128, dtype=np.float32)
    in_maps = []
    for b in range(B):
        P = np.ascontiguousarray(unit_pos[b * NU:(b + 1) * NU]).astype(
            np.float32, copy=False
        )
        n64 = (P.astype(np.float64) ** 2).sum(axis=1)
        ah, al = _split16((2.0 * P.astype(np.float64)).astype(np.float32))
        yh, yl = _split16(P)
        nh = n64.astype(np.float16)
        nl = (n64 - nh.astype(np.float64)).astype(np.float16)

        lhs = np.empty((KC, NPER * U), np.float16)  # u-major, reordered below
        for u in range(U):
            sl = slice(u * NPER, (u + 1) * NPER)
            lhs[0:3, sl] = ah[u::U].T
            lhs[3:6, sl] = ah[u::U].T
            lhs[6:9, sl] = al[u::U].T
            lhs[9:12, sl] = al[u::U].T
            lhs[12, sl] = -nh[u::U]
            lhs[13, sl] = -nl[u::U]
            lhs[14, sl] = -1.0
            lhs[15, sl] = -1.0
        # reorder to [t, u, i]: col t*512 + u*128 + i  <-  u*512 + t*128 + i
        lhs_tu = lhs.reshape(KC, U, MT, 128).transpose(0, 2, 1, 3).reshape(KC, -1)

        ops = np.empty((KC, U * NPER + NU), np.float16)
        ops[:, :RHS0] = lhs_tu[:, :512]
        ops[:, LHS1:] = lhs_tu[:, 512:]
        rsl = slice(RHS0, LHS1)
        ops[0:3, rsl] = yh.T
        ops[3:6, rsl] = yl.T
        ops[6:9, rsl] = yh.T
        ops[9:12, rsl] = yl.T
        ops[12, rsl] = 1.0
        ops[13, rsl] = 1.0
        ops[14, rsl] = nh
        ops[15, rsl] = nl
        in_maps.append({"ops": ops, "ident": ident})
    return in_maps


def _run_device(in_maps, trace=False):
    from concourse.bass_utils import run_bass_kernel_spmd

    nc = _get_nc()
    res = run_bass_kernel_spmd(
        nc, in_maps, core_ids=list(range(B)), trace=trace
    )
    return res


def _row_topk_f64(unit_pos, b, rloc):
    """Fallback for a repaired row: recompute its d2min in float64 and take
    the top-K with reference tie semantics (ascending d2, ties by index)."""
    P = unit_pos[b * NU:(b + 1) * NU].astype(np.float64).reshape(NPER, U, 3)
    d = P[rloc][:, None, None, :] - P[None, :, :, :]          # [U, 512, U, 3]
    d2 = np.einsum('ujvd,ujvd->ujv', d, d).min(axis=(0, 2))   # [512]
    return np.argsort(d2, kind="stable")[:KTOP]


def _postprocess(results, row, col, unit_pos):
    """Map device top-k (local j, per segment) to (row_o, col_o, attr)."""
    row_mat = row.reshape(NBLK, NPER)
    col_mat = col.reshape(NBLK, NPER)
    row_o = np.empty((NBLK, KTOP), np.int32)
    col_o = np.empty((NBLK, KTOP), np.int32)
    for b in range(B):
        r = results[b]
        idx = r["out_idx"].reshape(NPER, KTOP).astype(np.int64)
        val = r["out_val"].reshape(NPER, KTOP)

        # Rows where a bitwise-equal score appears more than once in the
        # top-16 show duplicate indices (max_index returns the first
        # occurrence for every equal value); also guard against non-monotone
        # value order.  Neither occurs for this input distribution — if one
        # does, re-derive the row exactly on the host.
        dup = (np.sort(idx, axis=1)[:, 1:] == np.sort(idx, axis=1)[:, :-1]).any(axis=1)
        nonmono = (np.diff(val, axis=1) > 0).any(axis=1)
        for rloc in np.flatnonzero(dup | nonmono):
            idx[rloc] = _row_topk_f64(unit_pos, b, rloc)

        gr = slice(b * NPER, (b + 1) * NPER)
        rows_local = np.arange(NPER)[:, None]
        row_o[gr] = row_mat[gr][rows_local, idx]
        col_o[gr] = col_mat[gr][rows_local, idx]
    attr = np.zeros(NBLK * KTOP, np.int32)
    return row_o.reshape(-1), col_o.reshape(-1), attr


def kernel(unit_pos, row, col, unit2block, segment_ids, k):
    unit_pos = np.asarray(unit_pos, dtype=np.float32)
    row = np.asarray(row, dtype=np.int32)
    col = np.asarray(col, dtype=np.int32)
    assert int(k) == KTOP
    in_maps = _make_core_inputs(unit_pos)
    res = _run_device(in_maps, trace=False)
    return _postprocess(res.results, row, col, unit_pos)
